# revision 34
# baseline (speedup 1.0000x reference)
"""Multi-head causal attention (B=4, T=2048, D=1024, H=16, HS=64) on 8 TRN2 cores.

Sharding: tensor-parallel over heads (2 heads/core) for QKV+attention, then an
AllToAll redistributes per-head context to token-parallel layout for the output
projection (each core projects 1024 tokens with the full Wp).

v2 design (from baseline trace analysis):
  - Receiver-side softmax normalization: the AllToAll payload is the RAW
    attention numerator plus the denominator row (65 rows per head, the
    ones-augmented AV matmul output). The receiving core batches all 16
    denominators, reciprocates them on DVE (reciprocal_approx_fast) and
    broadcasts across partitions with one K=16 matmul per k-tile. This removes
    the per-chunk ACT Exp<->Reciprocal table thrash (32 x 1.3us loads) and the
    PE stall behind the old broadcast matmul.
  - Phase A (QKV projection) chunks are interleaved into the phase-B attention
    stream: phase B alone is ACT(exp)-bound, so the extra projection matmuls
    keep the PE dense and the HAM clock at 8/8.
  - V is projected directly into [token, head-dim] layout (x-tile stationary,
    Wv moving) - no PE transposes, single strided copy into the AV operand.
  - Scores for the two heads run concurrently on PE row-groups 0/64 (K=64
    tile_position concurrency).
  - AllToAll split into 4 quarter-buffers to shrink the unoverlapped tail;
    startup DMAs reordered (wq + x-stream first on sync, the rest on gpsimd).

Compute dtype bf16 with fp32 PSUM accumulation.
"""
import numpy as np

import concourse.bass as bass
import concourse.tile as tile
from concourse import bacc, mybir
from concourse.bass_utils import run_bass_kernel_spmd

f32 = mybir.dt.float32
bf16 = mybir.dt.bfloat16

B, D, H, HS = 4, 1024, 16, 64
N_CORES = 8
HPC = H // N_CORES          # heads per core
QC = 512                    # q-chunk width
KT = 128                    # k-tile width
ND = D // 128               # din tiles

DT_NAME = "bf16"
DEBUG_DUMP = False


def _np_dt(dt):
    import ml_dtypes
    return {f32: np.float32, bf16: ml_dtypes.bfloat16}[dt]


def build_nc(T=2048, dt_name=DT_NAME):
    DT = bf16
    BT = B * T
    SL = BT // N_CORES              # tokens per core in phase C
    NQC = T // QC                   # q-chunks per batch
    NTB = T // KT                   # k-tiles per batch
    NQS = 2                         # a2a half splits
    QHF = SL // NQS                 # half width (tokens)

    nc = bacc.Bacc("TRN2", target_bir_lowering=False, debug=False,
                   num_devices=N_CORES)
    assert NQS == 2

    xt_d = nc.dram_tensor("xt", [D, BT], DT, kind="ExternalInput").ap()
    wq_d = nc.dram_tensor("wq", [D, 128], DT, kind="ExternalInput").ap()
    wk_d = nc.dram_tensor("wk", [D, 128], DT, kind="ExternalInput").ap()
    wv_d = nc.dram_tensor("wv", [D, 128], DT, kind="ExternalInput").ap()
    wp_d = nc.dram_tensor("wp", [D, D], DT, kind="ExternalInput").ap()
    bp_d = nc.dram_tensor("bp", [D, 1], f32, kind="ExternalInput").ap()
    triu_d = nc.dram_tensor("triu", [128, 128], DT, kind="ExternalInput").ap()
    emat_d = nc.dram_tensor("emat", [64, ND * 128], DT,
                            kind="ExternalInput").ap()
    out_d = nc.dram_tensor("outT", [D, SL], f32, kind="ExternalOutput").ap()
    if DEBUG_DUMP:
        dbg_a2a_d = nc.dram_tensor("dbg_a2a", [2, N_CORES, 130, SL // 2], DT,
                                   kind="ExternalOutput").ap()
        dbg_den_d = nc.dram_tensor("dbg_den", [2, 16, SL // 2], f32,
                                   kind="ExternalOutput").ap()
        dbg_cxn_d = nc.dram_tensor("dbg_cxn", [2, ND, 128, SL // 2], DT,
                                   kind="ExternalOutput").ap()
        dbg_recb_d = nc.dram_tensor("dbg_recb", [2, 64, SL // 2], DT,
                                    kind="ExternalOutput").ap()
        dbg_rb_d = nc.dram_tensor("dbg_rb", [2, ND, 128, SL // 2], f32,
                                  kind="ExternalOutput").ap()

    EXP = mybir.ActivationFunctionType.Exp
    ISQ = 1.0 / np.sqrt(HS)

    with tile.TileContext(nc) as tc:
        with (
            tc.tile_pool(name="wts", bufs=1) as wts,
            tc.tile_pool(name="acts", bufs=1) as acts,
            tc.tile_pool(name="dram", bufs=1, space="DRAM") as dram,
            tc.tile_pool(name="pA", bufs=2) as pA,
            tc.tile_pool(name="pB", bufs=4) as pB,
            tc.tile_pool(name="stg", bufs=2) as stg,
            tc.tile_pool(name="phc", bufs=2) as phc,
            tc.tile_pool(name="psP", bufs=2, space="PSUM") as psP,
            tc.tile_pool(name="psS", bufs=2, space="PSUM") as psS,
            tc.tile_pool(name="psV", bufs=1, space="PSUM") as psV,
        ):
            # ---- the x-stream owns the sync queue; all weights go through
            # gpsimd so their descriptor gen runs in parallel ----
            wq_sb, wk_sb, wv_sb = [], [], []
            for lst, dd, nm in ((wq_sb, wq_d, "wq"), (wk_sb, wk_d, "wk"),
                                (wv_sb, wv_d, "wv")):
                for j in range(ND):
                    t = wts.tile([128, 128], DT, name=f"{nm}{j}",
                                 tag=f"{nm}{j}")
                    nc.gpsimd.dma_start(t[:], dd[j * 128:(j + 1) * 128, :])
                    lst.append(t)
            triu_sb = wts.tile([128, 128], DT, name="triu", tag="triu")
            nc.gpsimd.dma_start(triu_sb[:], triu_d[:])
            emat_sb = wts.tile([64, ND * 128], DT, name="emat", tag="emat")
            nc.gpsimd.dma_start(emat_sb[:], emat_d[:])
            wp_sb = []
            for j in range(ND):
                t = wts.tile([128, D], DT, name=f"wp{j}", tag=f"wp{j}")
                nc.gpsimd.dma_start(t[:], wp_d[j * 128:(j + 1) * 128, :])
                wp_sb.append(t)
            bp_sb = []
            for m in range(ND):
                t = wts.tile([128, 1], f32, name=f"bp{m}", tag=f"bp{m}")
                nc.gpsimd.dma_start(t[:], bp_d[m * 128:(m + 1) * 128, :])
                bp_sb.append(t)

            # a2a buffers: the odd-chunk half "1" travels as one 1MB op
            # (overlapped under g1); the final even-chunk half is split into
            # quarters "0a"/"0b" so phC compute pipelines with the link
            a2a_keys = {"1": QHF, "0a": QHF // 2, "0b": QHF // 2}
            a2a_in = {k: dram.tile([N_CORES, 130, w], DT, name=f"a2ai{k}")
                      for k, w in a2a_keys.items()}
            a2a_out = {k: dram.tile([N_CORES, 130, w], DT, name=f"a2ao{k}")
                       for k, w in a2a_keys.items()}
            # output-column base per buffer (within this core's SL tokens)
            a2a_col0 = {"1": QHF, "0a": 0, "0b": QHF // 2}
            # persistent zero-padded broadcast operands (rows 16-63 stay 0)
            recb_sb = {}
            for k, w in a2a_keys.items():
                t = wts.tile([64, w], DT, name=f"recb{k}", tag=f"recb{k}")
                nc.vector.memset(t[:], 0.0)
                recb_sb[k] = t

            # per-batch activation tensors
            qT, kT, vA = [], [], []
            for b in range(B):
                qT.append(acts.tile([128, T], DT, name=f"qT{b}", tag=f"qT{b}"))
                kT.append(acts.tile([128, T], DT, name=f"kT{b}", tag=f"kT{b}"))
                vA.append(acts.tile([128, NTB * 130], DT, name=f"vA{b}",
                                    tag=f"vA{b}"))
                v3 = vA[b][:].rearrange("p (t c) -> p t c", c=130)
                nc.vector.memset(v3[:, :, 64], 1.0)
                nc.vector.memset(v3[:, :, 129], 1.0)

            # ACT warmup: load the exp table before the copy stream starts so
            # no mid-kernel table swap ever happens (identity lives in the
            # exp set too).
            warm = wts.tile([1, 16], f32, name="warm", tag="warm")
            nc.scalar.activation(warm[:], triu_sb[0:1, 0:16], EXP, scale=1.0)

            def a_chunk(b, ch):
                i0 = b * T + ch * QC
                xts = []
                for j in range(ND):
                    t = pA.tile([128, QC], DT, name=f"x{j}", tag=f"x{j}",
                                bufs=3)
                    nc.sync.dma_start(
                        t[:], xt_d[j * 128:(j + 1) * 128, i0:i0 + QC])
                    xts.append(t)
                sl = slice(ch * QC, (ch + 1) * QC)
                for w_sb, dst in ((wq_sb, qT[b]), (wk_sb, kT[b])):
                    pp = psP.tile([128, QC], f32, name="pp", tag="proj")
                    for j in range(ND):
                        nc.tensor.matmul(pp[:], w_sb[j][:], xts[j][:],
                                         start=(j == 0), stop=(j == ND - 1))
                    nc.vector.tensor_copy(dst[:, sl], pp[:])
                # V direct to [t, (h,e)]: x-slice stationary, Wv moving
                vo = psP.tile([128, QC], f32, name="vo", tag="proj")
                for tq in range(QC // 128):
                    for j in range(ND):
                        nc.tensor.matmul(
                            vo[:, tq * 128:(tq + 1) * 128],
                            xts[j][:, tq * 128:(tq + 1) * 128], wv_sb[j][:],
                            start=(j == 0), stop=(j == ND - 1))
                for tq in range(QC // 128):
                    slot = (ch * (QC // 128) + tq) * 130
                    dst3 = vA[b][:, slot:slot + 130].rearrange(
                        "p (h c) -> p h c", c=65)[:, :, 0:64]
                    src3 = vo[:, tq * 128:(tq + 1) * 128].rearrange(
                        "p (h c) -> p h c", c=64)
                    nc.vector.tensor_copy(dst3, src3)

            def b_chunk(b, qc):
                avs = [psV.tile([65, QC], f32, name=f"av{h}", tag=f"av{h}",
                                bufs=1) for h in range(HPC)]
                nj = 4 * qc + 4
                for j in range(nj):
                    jr = j - 4 * qc
                    off = max(jr, 0) * 128
                    w = QC - off
                    qsl = slice(qc * QC + off, (qc + 1) * QC)
                    sc = psS.tile([128, 2 * QC], f32, name="scb", tag="scb",
                                  bufs=2)
                    for h in range(HPC):
                        hp = slice(h * 64, (h + 1) * 64)
                        nc.tensor.matmul(
                            sc[:, h * QC:h * QC + w],
                            kT[b][hp, j * 128:(j + 1) * 128],
                            qT[b][hp, qsl], start=True, stop=True)
                    e = pB.tile([128, 2 * w], DT, name="exb", tag="exb",
                                bufs=4)
                    sc3 = sc[:].rearrange("p (two q) -> p two q",
                                          two=2)[:, :, 0:w]
                    e3 = e[:].rearrange("p (two q) -> p two q", two=2)
                    nc.scalar.activation(e3, sc3, EXP, scale=ISQ)
                    if jr >= 0:
                        for h in range(HPC):
                            nc.vector.tensor_mul(
                                e[:, h * w:h * w + 128],
                                e[:, h * w:h * w + 128], triu_sb[:])
                    for h in range(HPC):
                        lhs = vA[b][:, j * 130 + h * 65:j * 130 + h * 65 + 65]
                        nc.tensor.matmul(avs[h][:, off:QC], lhs,
                                         e[:, h * w:(h + 1) * w],
                                         start=(j == 0), stop=(j == nj - 1))
                # raw numerator + denominator row out to the a2a buffers
                d = 2 * b + qc // 2
                for h in range(HPC):
                    st = stg.tile([65, QC], DT, name=f"avs{h}", tag=f"avs{h}",
                                  bufs=2)
                    nc.vector.tensor_copy(st[:], avs[h][:])
                    if qc % 2 == 1:
                        nc.sync.dma_start(
                            a2a_in["1"][d, h * 65:(h + 1) * 65, :], st[:])
                    else:
                        hw_ = QHF // 2
                        nc.sync.dma_start(
                            a2a_in["0a"][d, h * 65:(h + 1) * 65, :],
                            st[:, 0:hw_])
                        nc.sync.dma_start(
                            a2a_in["0b"][d, h * 65:(h + 1) * 65, :],
                            st[:, hw_:QHF])

            def a2a(k):
                nc.gpsimd.collective_compute(
                    "AllToAll", mybir.AluOpType.bypass,
                    replica_groups=[list(range(N_CORES))],
                    ins=[a2a_in[k].opt()], outs=[a2a_out[k].opt()])

            cq_state = {}

            def c_load(k):
                ao = a2a_out[k]
                w = a2a_keys[k]
                den = phc.tile([16, w], DT, name="den", tag="den", bufs=2)
                nc.gpsimd.dma_start(
                    den[:],
                    ao[:].rearrange("j (h r) t -> (j h) r t", r=65)[:, 64])
                # keep the chain on gpsimd (same FIFO as the gather DMA - no
                # head-of-line stall on ACT/DVE mid-stream)
                denf = phc.tile([16, w], f32, name="denf", tag="denf", bufs=2)
                nc.gpsimd.tensor_copy(denf[:], den[:])
                rec = phc.tile([16, w], f32, name="rec", tag="rec", bufs=2)
                nc.vector.reciprocal_approx_fast(rec[:], denf[:])
                # K=64 broadcast operand: reciprocals into rows 0-15 of the
                # persistent zero-padded tile (K=16 matmuls mis-drive the PE
                # col-groups here, so pad to the proven 64x128 config)
                recb = recb_sb[k]
                nc.gpsimd.tensor_copy(recb[0:16, :], rec[:])
                cxs = []
                for j in range(ND):
                    cx = phc.tile([128, w], DT, name=f"cx{j}",
                                  tag=f"cx{j}", bufs=2)
                    # two contiguous per-head DMAs: a partition-split dst AP
                    # silently drops all but the first row per group
                    nc.gpsimd.dma_start(cx[0:64, :], ao[j][0:64, :])
                    nc.gpsimd.dma_start(cx[64:128, :], ao[j][65:129, :])
                    cxs.append(cx)
                cq_state[k] = (recb, cxs)

            def c_compute(k):
                recb, cxs = cq_state.pop(k)
                w = a2a_keys[k]
                col0 = a2a_col0[k]
                cxn = []
                for j in range(ND):
                    rb = psP.tile([128, w], f32, name="rb", tag="proj")
                    nc.tensor.matmul(rb[:], emat_sb[:, j * 128:(j + 1) * 128],
                                     recb[:], start=True, stop=True)
                    cn = phc.tile([128, w], DT, name=f"cxn{j}",
                                  tag=f"cxn{j}", bufs=2)
                    nc.vector.tensor_mul(cn[:], cxs[j][:], rb[:])
                    cxn.append(cn)
                for m in range(ND):
                    op = psP.tile([128, w], f32, name="op", tag="proj")
                    for j in range(ND):
                        nc.tensor.matmul(
                            op[:], wp_sb[j][:, m * 128:(m + 1) * 128],
                            cxn[j][:], start=(j == 0), stop=(j == ND - 1))
                    os_ = phc.tile([128, w], f32, name="os", tag="os",
                                   bufs=2)
                    nc.vector.tensor_scalar_add(os_[:], op[:], bp_sb[m][:])
                    nc.scalar.dma_start(
                        out_d[m * 128:(m + 1) * 128, col0:col0 + w], os_[:])

            # ---- emission schedule: odd q-chunks (heavier attention) run
            # with the interleaved projections; light evens in g1 ----
            with nc.named_scope("g0"):
                for ch in range(NQC):
                    a_chunk(0, ch)
                g0 = [("B", 0, 1), ("A", 1, 0), ("A", 1, 1),
                      ("B", 0, 3), ("A", 1, 2), ("A", 1, 3),
                      ("B", 1, 1), ("A", 2, 0), ("A", 2, 1),
                      ("B", 1, 3), ("A", 2, 2), ("A", 2, 3),
                      ("B", 2, 1), ("A", 3, 0), ("A", 3, 1),
                      ("B", 2, 3), ("A", 3, 2), ("A", 3, 3),
                      ("B", 3, 1), ("B", 3, 3)]
                for kind, b, i in g0:
                    (b_chunk if kind == "B" else a_chunk)(b, i)
            a2a("1")
            with nc.named_scope("g1"):
                b_chunk(0, 0)
                b_chunk(0, 2)
                b_chunk(1, 0)
                b_chunk(1, 2)
                b_chunk(2, 0)
                c_load("1")
                b_chunk(2, 2)
                b_chunk(3, 0)
                b_chunk(3, 2)
            a2a("0a")
            a2a("0b")
            with nc.named_scope("tail"):
                c_compute("1")
                c_load("0a")
                c_compute("0a")
                c_load("0b")
                c_compute("0b")

    nc.compile()
    return nc


def prep_inputs(x, Wq, Wk, Wv, Wp, bp, T, dt_name=DT_NAME):
    """Host-side sharding/layout prep. Returns in_maps for the 8 cores."""
    ndt = _np_dt(bf16)
    BT = B * T

    x = np.asarray(x, np.float32)
    Wq = np.asarray(Wq, np.float32)
    Wk = np.asarray(Wk, np.float32)
    Wv = np.asarray(Wv, np.float32)
    Wp = np.asarray(Wp, np.float32)
    bp = np.asarray(bp, np.float32)

    xt = np.ascontiguousarray(x.reshape(BT, D).T).astype(ndt)
    wp = np.ascontiguousarray(Wp.T).astype(ndt)
    bpc = np.ascontiguousarray(bp.reshape(D, 1))
    triu = np.triu(np.ones((128, 128), np.float32)).astype(ndt)
    # emat[h, j*128 + p] = 1 iff h == 2j + p//64  (partition-broadcast of the
    # 16 denominator reciprocals onto the phase-C k-tile layout); padded to
    # K=64 so the PE runs in the proven (64,128) tile config
    emat = np.zeros((64, ND * 128), np.float32)
    for j in range(ND):
        for p in range(128):
            emat[2 * j + p // 64, j * 128 + p] = 1.0
    emat = emat.astype(ndt)

    def wslice(W, c):
        # [H, D, HS] heads 2c,2c+1 -> [D, 128] as [d, (h_local, e)]
        return np.ascontiguousarray(
            W[2 * c:2 * c + 2].transpose(1, 0, 2).reshape(D, 2 * HS)).astype(ndt)

    in_maps = []
    for c in range(N_CORES):
        in_maps.append({
            "xt": xt, "wq": wslice(Wq, c), "wk": wslice(Wk, c),
            "wv": wslice(Wv, c), "wp": wp, "bp": bpc,
            "triu": triu, "emat": emat,
        })
    return in_maps


_NC_CACHE = {}


def kernel(x, Wq, Wk, Wv, Wp, bp):
    T = np.asarray(x).shape[1]
    key = (T, DT_NAME)
    if key not in _NC_CACHE:
        _NC_CACHE[key] = build_nc(T, DT_NAME)
    nc = _NC_CACHE[key]
    in_maps = prep_inputs(x, Wq, Wk, Wv, Wp, bp, T, DT_NAME)
    res = run_bass_kernel_spmd(nc, in_maps, list(range(N_CORES)))
    out = np.concatenate([res.results[c]["outT"].T for c in range(N_CORES)],
                         axis=0)
    return np.ascontiguousarray(out.reshape(B, T, D).astype(np.float32))
